# revision 18
# baseline (speedup 1.0000x reference)
"""Trainium2 Bass kernel for a GPT-style causal attention block.

  y = proj( softmax_causal( (x@Wq)(x@Wk)^T / sqrt(hd) ) @ (x@Wv) )

Shapes (hardcoded): B=2, S=2048, D=1024, H=16 heads, hd=64.

Sharding over 8 NeuronCores: core = (batch b, head-group g), g selects 4
heads (2 head PAIRS). Single SPMD program; per core:

  for ib in 0..3 (512-query slices; causal, so slice ib attends to
                  keys [0, 512*(ib+1)) ):
      qkT projection for query block ib (chases the x DMA, which lands
          query-block by query-block), v projection for key tiles
          4ib..4ib+3
      deferred normalize + AllGather of the previous slice's pair-1
      attention(ib, pair0), attention(ib, pair1):
          scores in the transposed [key, query] layout, head pairs
          packed into PE row groups; exp on ACT (1/8 scale folded in);
          causal masking multiplies only the [128,128] triangle of each
          diagonal key tile on the GPSIMD engine, and exp/AV are
          restricted to the unmasked query range (full width on ib=0);
          AV with lhsT=[v|1] so psum row 64 accumulates the softmax
          denominator.
      pair0's normalize (DVE reciprocal of the denominator row, PE
          replicate matmul, multiply) is EMBEDDED into pair1's jt loop
          and pair1's into the next iteration's qkT block, so the
          reciprocal latency never stalls the in-order PE queue. Each
          pair's 128-row aT slice is AllGather'd separately (8 small
          collectives total), overlapping the next compute.
      proj(ib-1): output projection of the previous gathered slice,
          transposed (lhsT = Wp tile, 512-moving), accumulating the two
          gathered halves; bias is a per-partition scalar add.

All matmul operands are bf16 (halves PE weight-load time; bf16 moving
operands always run at full PE rate). PSUM accumulation is f32.
Measured relative L2 error vs the fp32 reference: ~4e-3.
"""

import numpy as np

B = 2
S = 2048
D = 1024
H = 16
HD = 64
HLOC = 4          # heads per core
NPAIR = 2         # head pairs per core
N_CORES = 8
GROUP = 4         # cores per batch (replica group size)
IB = 512          # query block width (matmul moving dim)
OC = D // GROUP   # output-projection column shard per core (256)
SCALE = 1.0 / 8.0  # 1/sqrt(hd)


def _build_bass(s=S):
    import concourse.bacc as bacc
    import concourse.mybir as mybir
    import concourse.tile as tile

    f32 = mybir.dt.float32
    f32r = mybir.dt.float32r
    bf16 = mybir.dt.bfloat16
    Alu = mybir.AluOpType
    Act = mybir.ActivationFunctionType

    n_ib = s // IB           # query blocks (4)
    n_st = s // 128          # 128-row sequence tiles (16)
    n_dt = D // 128          # contraction tiles for D (8)

    nc = bacc.Bacc(num_devices=N_CORES)

    xt = nc.declare_dram_parameter("xt", [D, s], bf16, isOutput=False)
    wqk = nc.declare_dram_parameter("wqk", [D, 512], bf16, isOutput=False)
    wv = nc.declare_dram_parameter("wv", [D, 256], bf16, isOutput=False)
    bqk = nc.declare_dram_parameter("bqk", [128, 4], f32, isOutput=False)
    bv = nc.declare_dram_parameter("bv", [128, 256], f32, isOutput=False)
    wp = nc.declare_dram_parameter("wp", [D, OC], bf16, isOutput=False)
    bpt = nc.declare_dram_parameter("bpt", [128, 2], f32, isOutput=False)
    # full-width causal masks for ib=0 (mskf[j,k,:,i] = i >= j + 128k) and
    # the [128,128] triangle (mskt[j,:,c] = c >= j) for diagonal tiles
    mskf = nc.declare_dram_parameter("mskf", [128, 4, 2, IB], bf16, isOutput=False)
    mskt = nc.declare_dram_parameter("mskt", [128, 2, 128], bf16, isOutput=False)
    # selector row for the reciprocal replicate matmul (all-ones [1, 64])
    sel2 = nc.declare_dram_parameter("sel2", [1, 64], f32r, isOutput=False)
    y = nc.declare_dram_parameter("y", [OC, s], f32, isOutput=True)

    with tile.TileContext(nc) as tc:
        with (
            tc.tile_pool(name="const", bufs=1) as const,
            tc.tile_pool(name="persist", bufs=1) as persist,
            tc.tile_pool(name="dram", bufs=1, space="DRAM") as dram,
            tc.tile_pool(name="ps_s", bufs=2, space="PSUM") as ps_s,
            tc.tile_pool(name="ps_av", bufs=2, space="PSUM") as ps_av,
            tc.tile_pool(name="pt", bufs=4) as ptpool,
            tc.tile_pool(name="small", bufs=4) as small,
            tc.tile_pool(name="agf", bufs=4) as agfpool,
            tc.tile_pool(name="yout", bufs=2) as yout,
        ):
            # ---- weights first so the QKV matmuls can start ASAP; x
            # arrives query-block by query-block, interleaved with the
            # tensors each upcoming phase needs next ----
            wqk_sb = const.tile([128, n_dt, 512], bf16)
            nc.sync.dma_start(
                out=wqk_sb, in_=wqk.rearrange("(t p) c -> p t c", p=128)
            )
            bqk_sb = const.tile([128, 4], f32)
            nc.sync.dma_start(out=bqk_sb, in_=bqk[:, :])
            xt_sb = const.tile([128, n_dt, s], bf16)
            xt_r = xt.rearrange("(t p) ss -> p t ss", p=128)

            def load_x(sh):
                for t in range(n_dt):
                    nc.sync.dma_start(
                        out=xt_sb[:, t, sh * IB : (sh + 1) * IB],
                        in_=xt_r[:, t, sh * IB : (sh + 1) * IB],
                    )

            wv_sb = const.tile([128, n_dt, 256], bf16)
            nc.sync.dma_start(
                out=wv_sb, in_=wv.rearrange("(t p) c -> p t c", p=128)
            )
            bv_sb = const.tile([128, 256], f32)
            nc.sync.dma_start(out=bv_sb, in_=bv[:, :])
            load_x(0)
            mskt_sb = const.tile([128, 2, 128], bf16)
            nc.sync.dma_start(out=mskt_sb, in_=mskt[:, :, :])
            mskf_sb = const.tile([128, 4, 2, IB], bf16)
            nc.sync.dma_start(out=mskf_sb, in_=mskf[:, :, :, :])
            sel2_sb = const.tile([1, 64], f32r)
            nc.sync.dma_start(out=sel2_sb, in_=sel2[:, :])
            load_x(1)
            wp_sb = const.tile([128, n_dt, OC], bf16)
            nc.sync.dma_start(
                out=wp_sb, in_=wp.rearrange("(t p) c -> p t c", p=128)
            )
            bpt_sb = const.tile([128, 2], f32)
            nc.sync.dma_start(out=bpt_sb, in_=bpt[:, :])
            load_x(2)
            load_x(3)

            # tiny dummy AllGather to absorb the first-collective channel
            # warmup (~20-30us) while the input DMA streams in
            agw_in = dram.tile([128, 4], bf16, name="agw_in")
            agw_out = dram.tile([512, 4], bf16, name="agw_out")
            nc.sync.dma_start(out=agw_in[:, :], in_=mskt[:, 0, 0:4])
            nc.gpsimd.collective_compute(
                "AllGather",
                Alu.bypass,
                replica_groups=[[0, 1, 2, 3], [4, 5, 6, 7]],
                ins=[agw_in[:, :]],
                outs=[agw_out[:, :]],
            )

            # dummy exp: pulls the ACT exp table load off the critical path
            warm_sb = const.tile([1, 1], f32)
            nc.scalar.activation(
                out=warm_sb, in_=bqk_sb[0:1, 0:1], func=Act.Exp, scale=0.0
            )

            # persistent intermediates (all bf16)
            qT_sb = persist.tile([128, NPAIR, s], bf16)   # [pair_ch, pair, s]
            kT_sb = persist.tile([128, NPAIR, s], bf16)
            v_sb = persist.tile([128, n_st, HLOC, 65], bf16)
            aT_sb = persist.tile([128, NPAIR, s], bf16)

            # ones column for the softmax-denominator trick
            nc.gpsimd.memset(v_sb[:, :, :, 64:65], 1.0)

            ag_in = [
                [dram.tile([128, IB], bf16, name=f"ag_in{i}_{p}") for p in range(2)]
                for i in range(n_ib)
            ]
            ag_out = [
                [dram.tile([512, IB], bf16, name=f"ag_out{i}_{p}") for p in range(2)]
                for i in range(n_ib)
            ]

            def qkT_v(ib):
                # c-tile t: 0,1 = q pair0/1; 2,3 = k pair0/1
                for t in range(4):
                    ps = ps_s.tile([128, IB], f32, name="ps", tag="pss")
                    for dt in range(n_dt):
                        nc.tensor.matmul(
                            ps,
                            lhsT=(wqk_sb[:, dt, t * 128 : (t + 1) * 128]),
                            rhs=(xt_sb[:, dt, ib * IB : (ib + 1) * IB]),
                            start=(dt == 0),
                            stop=(dt == n_dt - 1),
                        )
                    dst = qT_sb if t < 2 else kT_sb
                    nc.vector.tensor_scalar_add(
                        out=dst[:, t % 2, ib * IB : (ib + 1) * IB],
                        in0=ps,
                        scalar1=bqk_sb[:, t : t + 1],
                    )
                for st in range(4 * ib, 4 * ib + 4):
                    psv = ps_s.tile([128, 256], f32, name="psv", tag="pss")
                    for dt in range(n_dt):
                        nc.tensor.matmul(
                            psv,
                            lhsT=(xt_sb[:, dt, st * 128 : (st + 1) * 128]),
                            rhs=(wv_sb[:, dt, :]),
                            start=(dt == 0),
                            stop=(dt == n_dt - 1),
                        )
                    nc.vector.tensor_tensor(
                        out=v_sb[:, st, :, 0:64],
                        in0=psv.rearrange("p (h e) -> p h e", h=HLOC),
                        in1=bv_sb.rearrange("p (h e) -> p h e", h=HLOC),
                        op=Alu.add,
                    )

            def norm_recips(st_):
                """DVE reciprocals of the two denominator rows (start early)."""
                ib, pair, avs = st_["ib"], st_["pair"], st_["avs"]
                for hh in range(2):
                    rec1 = small.tile([1, IB], f32r, name="rec1")
                    with nc.allow_low_precision(
                        reason="float32r feeds the fp32r replicate matmul"
                    ):
                        nc.vector.reciprocal(out=rec1, in_=avs[hh][64:65, :])
                    st_["rec1"].append(rec1)

            def norm_repmm(st_, hh):
                """PE replicate matmul for one head (emit as PE filler)."""
                rec_ps = ps_s.tile([64, IB], f32, name="rec_ps", tag="pss")
                nc.tensor.matmul(
                    rec_ps,
                    lhsT=(sel2_sb[0:1, 0:64]),
                    rhs=(st_["rec1"][hh][:, :]),
                    start=True,
                    stop=True,
                )
                st_["rec_ps"].append(rec_ps)

            def norm_finish(st_):
                """Copies + multiplies + stage + collective for one pair."""
                ib, pair, avs = st_["ib"], st_["pair"], st_["avs"]
                rec_rep = small.tile([128, IB], f32, name="rec_rep")
                for hh in range(2):
                    nc.vector.tensor_copy(
                        out=rec_rep[hh * 64 : (hh + 1) * 64, :],
                        in_=st_["rec_ps"][hh],
                    )
                for hh in range(2):
                    poff = hh * 64
                    nc.vector.tensor_tensor(
                        out=aT_sb[poff : poff + 64, pair, ib * IB : (ib + 1) * IB],
                        in0=avs[hh][0:64, :],
                        in1=rec_rep[poff : poff + 64, :],
                        op=Alu.mult,
                    )
                nc.sync.dma_start(
                    out=ag_in[ib][pair][:, :],
                    in_=aT_sb[:, pair, ib * IB : (ib + 1) * IB],
                )
                nc.gpsimd.collective_compute(
                    "AllGather",
                    Alu.bypass,
                    replica_groups=[[0, 1, 2, 3], [4, 5, 6, 7]],
                    ins=[ag_in[ib][pair][:, :]],
                    outs=[ag_out[ib][pair][:, :]],
                )

            def att_jt(ib, pair, jseq, jt, njt, avs, pend):
                """Emit one jt step (scores, exp, mask, prev AV) for one
                pair; returns the new pending AV pair."""
                k = jt - 4 * ib
                off = 128 * k if (ib > 0 and k >= 1) else 0
                pss = ps_s.tile([128, 2, IB], f32, name="pss", tag="pss")
                for hh in range(2):
                    poff = hh * 64
                    nc.tensor.matmul(
                        pss[:, hh, off:],
                        lhsT=(kT_sb[
                                poff : poff + 64, pair,
                                jt * 128 : (jt + 1) * 128,
                            ]),
                        rhs=(qT_sb[
                                poff : poff + 64, pair,
                                ib * IB + off : (ib + 1) * IB,
                            ]),
                        start=True,
                        stop=True,
                        tile_position=(poff, 0),
                    )
                pt = ptpool.tile([128, 2, IB], bf16, name="pt")
                nc.scalar.activation(
                    out=pt[:, :, off:], in_=pss[:, :, off:],
                    func=Act.Exp, scale=SCALE,
                )
                if k >= 0:  # diagonal tile: causal mask (on gpsimd)
                    if ib > 0 or k == 0:
                        nc.gpsimd.tensor_tensor(
                            out=pt[:, :, off : off + 128],
                            in0=pt[:, :, off : off + 128],
                            in1=mskt_sb,
                            op=Alu.mult,
                        )
                    else:  # ib == 0, k >= 1: full-width mask
                        nc.gpsimd.tensor_tensor(
                            out=pt, in0=pt, in1=mskf_sb[:, k, :, :],
                            op=Alu.mult,
                        )
                if pend is not None:
                    for mm in pend:
                        nc.tensor.matmul(**mm)
                return [
                    dict(
                        out=avs[hh][:, off:],
                        lhsT=(v_sb[:, jt, pair * 2 + hh, :]),
                        rhs=(pt[:, hh, off:]),
                        start=(jseq == 0),
                        stop=(jseq == njt - 1),
                    )
                    for hh in range(2)
                ]

            def attention_both(ib):
                """Both head pairs' jt streams interleaved: one pipeline
                fill per query block instead of two."""
                njt = 4 * (ib + 1)
                jt_order = list(range(4 * ib, njt)) + list(range(4 * ib))
                sts, avs, pend = [], {}, {}
                for pair in range(2):
                    avs[pair] = [
                        ps_av.tile([65, IB], f32, name=f"av{pair}{hh}", tag=f"av{hh}")
                        for hh in range(2)
                    ]
                    pend[pair] = None
                for jseq, jt in enumerate(jt_order):
                    for pair in range(2):
                        pend[pair] = att_jt(
                            ib, pair, jseq, jt, njt, avs[pair], pend[pair]
                        )
                for pair in range(2):
                    for mm in pend[pair]:
                        nc.tensor.matmul(**mm)
                    st_ = dict(ib=ib, pair=pair, avs=avs[pair], rec1=[], rec_ps=[])
                    norm_recips(st_)
                    sts.append(st_)
                return sts

            def attention(ib, pair):
                """Single-pair jt stream (used for the final query block so
                pair0's gather can fire during pair1's attention)."""
                njt = 4 * (ib + 1)
                avs = [
                    ps_av.tile([65, IB], f32, name=f"av{hh}", tag=f"av{hh}")
                    for hh in range(2)
                ]
                jt_order = list(range(4 * ib, njt)) + list(range(4 * ib))
                pend = None
                for jseq, jt in enumerate(jt_order):
                    pend = att_jt(ib, pair, jseq, jt, njt, avs, pend)
                for mm in pend:
                    nc.tensor.matmul(**mm)
                st_ = dict(ib=ib, pair=pair, avs=avs, rec1=[], rec_ps=[])
                norm_recips(st_)
                return st_

            def norm_pe(st_):
                for hh in range(2):
                    norm_repmm(st_, hh)
                norm_finish(st_)

            def proj_slice(ib):
                """Output projection for gathered slice ib:
                yT[oc, q] = sum_c Wp[c, oc] agT[c, q], accumulating the
                two gathered pair-halves."""
                agfs = []
                for part in range(2):
                    agf_sb = agfpool.tile(
                        [128, 4, IB], bf16, name=f"agf{part}", tag="agf"
                    )
                    nc.sync.dma_start(
                        out=agf_sb,
                        in_=ag_out[ib][part].rearrange("(t p) q -> p t q", p=128),
                    )
                    agfs.append(agf_sb)
                psy = [
                    ps_s.tile([128, IB], f32, name=f"psy{h}", tag="pss")
                    for h in range(2)
                ]
                for part in range(2):
                    for h in range(2):
                        for g in range(4):
                            nc.tensor.matmul(
                                psy[h],
                                lhsT=(wp_sb[:, 2 * g + part, h * 128 : (h + 1) * 128]),
                                rhs=(agfs[part][:, g, :]),
                                start=(part == 0 and g == 0),
                                stop=(part == 1 and g == 3),
                            )
                for h in range(2):
                    ysb = yout.tile([128, IB], f32, name="ysb")
                    nc.vector.tensor_scalar_add(
                        out=ysb, in0=psy[h], scalar1=bpt_sb[:, h : h + 1]
                    )
                    nc.sync.dma_start(
                        out=y[h * 128 : (h + 1) * 128, ib * IB : (ib + 1) * IB],
                        in_=ysb,
                    )

            # Iterations 0..n-2: pairs interleaved; both normalizes deferred
            # behind the next iteration's qkT/v block (the reciprocals run
            # during it, so the PE replicate matmuls never stall).
            # Final iteration: pairs sequential so pair0's gather flies
            # during pair1's attention; proj of the previous slice fills
            # the reciprocal latency.
            pendings = []
            for ib in range(n_ib - 1):
                qkT_v(ib)
                for st_ in pendings:
                    norm_pe(st_)
                pendings = attention_both(ib)
                if ib > 0:
                    proj_slice(ib - 1)
            ib = n_ib - 1
            qkT_v(ib)
            for st_ in pendings:
                norm_pe(st_)
            st0 = attention(ib, 0)
            proj_slice(ib - 1)      # fills pair0's reciprocal latency
            norm_pe(st0)            # fires AG(last, pair0) before att pair1
            st1 = attention(ib, 1)
            norm_pe(st1)
            proj_slice(ib)

    nc.compile()
    return nc


def _shard_inputs(x, w_attn, b_attn, w_proj, b_proj, s=S):
    """Host-side sharding: build the per-core input maps."""
    import ml_dtypes
    bf16 = ml_dtypes.bfloat16
    x = np.asarray(x, dtype=np.float32)
    w_attn = np.asarray(w_attn, dtype=np.float32)
    b_attn = np.asarray(b_attn, dtype=np.float32)
    w_proj = np.asarray(w_proj, dtype=np.float32)
    b_proj = np.asarray(b_proj, dtype=np.float32)

    # full-width causal mask tiles for ib=0: mskf[j, k, :, i] = 1.0 if i >= j+128k
    jj = np.arange(128)[:, None, None, None]
    kk = np.arange(4)[None, :, None, None]
    ii = np.arange(IB)[None, None, None, :]
    mskf = np.broadcast_to(ii >= jj + 128 * kk, (128, 4, 2, IB)).astype(bf16)
    # triangle mask (duplicated for the 2-head layout)
    mskt = np.broadcast_to(
        (np.arange(128)[None, None, :] >= np.arange(128)[:, None, None]),
        (128, 2, 128),
    ).astype(bf16)

    in_maps = []
    for core in range(N_CORES):
        b, g = divmod(core, GROUP)
        hs = list(range(g * HLOC, (g + 1) * HLOC))
        xt = np.ascontiguousarray(x[b].T).astype(bf16)
        qcols = np.concatenate(
            [w_attn[:, h * HD : (h + 1) * HD] for h in hs], axis=1
        )
        kcols = np.concatenate(
            [w_attn[:, D + h * HD : D + (h + 1) * HD] for h in hs], axis=1
        )
        vcols = np.concatenate(
            [w_attn[:, 2 * D + h * HD : 2 * D + (h + 1) * HD] for h in hs], axis=1
        )
        wqk = np.ascontiguousarray(
            np.concatenate([qcols, kcols], axis=1)
        ).astype(bf16)
        wv = np.ascontiguousarray(vcols).astype(bf16)
        bq = np.concatenate([b_attn[h * HD : (h + 1) * HD] for h in hs])
        bk = np.concatenate([b_attn[D + h * HD : D + (h + 1) * HD] for h in hs])
        bvv = np.concatenate(
            [b_attn[2 * D + h * HD : 2 * D + (h + 1) * HD] for h in hs]
        )
        bqk = np.concatenate([bq, bk]).reshape(4, 128).T.copy()  # [128, 4]
        bv = np.broadcast_to(bvv, (128, 256)).copy()
        wpc = np.ascontiguousarray(w_proj[:, g * OC : (g + 1) * OC]).astype(bf16)
        bpt = np.ascontiguousarray(
            b_proj[g * OC : (g + 1) * OC].reshape(2, 128).T
        )  # bpt[p, h] = b_proj[g*256 + 128h + p]
        in_maps.append(
            dict(
                xt=xt, wqk=wqk, wv=wv, bqk=bqk, bv=bv, wp=wpc, bpt=bpt,
                mskf=mskf, mskt=mskt, sel2=np.ones((1, 64), np.float32),
            )
        )
    return in_maps


def _unshard(results):
    y = np.empty((B, S, D), np.float32)
    for core in range(N_CORES):
        b, g = divmod(core, GROUP)
        y[b, :, g * OC : (g + 1) * OC] = results[core]["y"].T
    return y


_NC_CACHE = {}


def kernel(x, w_attn, b_attn, w_proj, b_proj):
    from concourse.bass_utils import run_bass_kernel_spmd

    if S not in _NC_CACHE:
        _NC_CACHE[S] = _build_bass(S)
    nc = _NC_CACHE[S]
    in_maps = _shard_inputs(x, w_attn, b_attn, w_proj, b_proj)
    res = run_bass_kernel_spmd(nc, in_maps, list(range(N_CORES)))
    return _unshard(res.results)


# revision 27
# speedup vs baseline: 1.0314x; 1.0314x over previous
"""Trainium2 Bass kernel for a GPT-style causal attention block.

  y = proj( softmax_causal( (x@Wq)(x@Wk)^T / sqrt(hd) ) @ (x@Wv) )

Shapes (hardcoded): B=2, S=2048, D=1024, H=16 heads, hd=64.

Sharding over 8 NeuronCores: core = (batch b, head-group g), g selects 4
heads (2 head PAIRS). Single SPMD program; per core:

  for ib in 0..3 (512-query slices; causal, so slice ib attends to
                  keys [0, 512*(ib+1)) ):
      qkT projection for query block ib (chases the x DMA, which lands
          query-block by query-block), v projection for key tiles
          4ib..4ib+3
      deferred normalize + AllGather of the previous slice's pair-1
      attention(ib, pair0), attention(ib, pair1):
          scores in the transposed [key, query] layout, head pairs
          packed into PE row groups; exp on ACT (1/8 scale folded in);
          causal masking multiplies only the [128,128] triangle of each
          diagonal key tile on the GPSIMD engine, and exp/AV are
          restricted to the unmasked query range (full width on ib=0);
          AV with lhsT=[v|1] so psum row 64 accumulates the softmax
          denominator.
      pair0's normalize (DVE reciprocal of the denominator row, PE
          replicate matmul, multiply) is EMBEDDED into pair1's jt loop
          and pair1's into the next iteration's qkT block, so the
          reciprocal latency never stalls the in-order PE queue. Each
          pair's 128-row aT slice is AllGather'd separately (8 small
          collectives total), overlapping the next compute.
      proj(ib-1): output projection of the previous gathered slice,
          transposed (lhsT = Wp tile, 512-moving), accumulating the two
          gathered halves; bias is a per-partition scalar add.

All matmul operands are bf16 (halves PE weight-load time; bf16 moving
operands always run at full PE rate). PSUM accumulation is f32.
Measured relative L2 error vs the fp32 reference: ~4e-3.
"""

import numpy as np

B = 2
S = 2048
D = 1024
H = 16
HD = 64
HLOC = 4          # heads per core
NPAIR = 2         # head pairs per core
N_CORES = 8
GROUP = 4         # cores per batch (replica group size)
IB = 512          # query block width (matmul moving dim)
OC = D // GROUP   # output-projection column shard per core (256)
SCALE = 1.0 / 8.0  # 1/sqrt(hd)


def _build_bass(s=S):
    import concourse.bacc as bacc
    import concourse.mybir as mybir
    import concourse.tile as tile

    f32 = mybir.dt.float32
    f32r = mybir.dt.float32r
    bf16 = mybir.dt.bfloat16
    Alu = mybir.AluOpType
    Act = mybir.ActivationFunctionType

    n_ib = s // IB           # query blocks (4)
    n_st = s // 128          # 128-row sequence tiles (16)
    n_dt = D // 128          # contraction tiles for D (8)

    nc = bacc.Bacc(num_devices=N_CORES)

    xt = nc.declare_dram_parameter("xt", [D, s], bf16, isOutput=False)
    wqk = nc.declare_dram_parameter("wqk", [D, 512], bf16, isOutput=False)
    wv = nc.declare_dram_parameter("wv", [D, 256], bf16, isOutput=False)
    bqk = nc.declare_dram_parameter("bqk", [128, 4], f32, isOutput=False)
    bv = nc.declare_dram_parameter("bv", [128, 256], f32, isOutput=False)
    wp = nc.declare_dram_parameter("wp", [D, OC], bf16, isOutput=False)
    bpt = nc.declare_dram_parameter("bpt", [128, 2], f32, isOutput=False)
    # full-width causal masks for ib=0 (mskf[j,k,:,i] = i >= j + 128k) and
    # the [128,128] triangle (mskt[j,:,c] = c >= j) for diagonal tiles
    mskf = nc.declare_dram_parameter("mskf", [128, 4, 2, IB], bf16, isOutput=False)
    mskt = nc.declare_dram_parameter("mskt", [128, 2, 128], bf16, isOutput=False)
    # selector row for the reciprocal replicate matmul (all-ones [1, 64])
    sel2 = nc.declare_dram_parameter("sel2", [1, 64], f32r, isOutput=False)
    y = nc.declare_dram_parameter("y", [OC, s], f32, isOutput=True)

    with tile.TileContext(nc) as tc:
        with (
            tc.tile_pool(name="const", bufs=1) as const,
            tc.tile_pool(name="persist", bufs=1) as persist,
            tc.tile_pool(name="dram", bufs=1, space="DRAM") as dram,
            tc.tile_pool(name="ps_s", bufs=2, space="PSUM") as ps_s,
            tc.tile_pool(name="ps_av", bufs=2, space="PSUM") as ps_av,
            tc.tile_pool(name="pt", bufs=4) as ptpool,
            tc.tile_pool(name="small", bufs=4) as small,
            tc.tile_pool(name="agf", bufs=4) as agfpool,
            tc.tile_pool(name="yout", bufs=2) as yout,
        ):
            # ---- weights first so the QKV matmuls can start ASAP; x
            # arrives query-block by query-block, interleaved with the
            # tensors each upcoming phase needs next ----
            # tiny dummy AllGather FIRST to absorb the first-collective
            # channel warmup (~20-30us) while the input DMA streams in
            agw_in = dram.tile([128, 4], bf16, name="agw_in")
            agw_out = dram.tile([512, 4], bf16, name="agw_out")
            nc.sync.dma_start(out=agw_in[:, :], in_=mskt[:, 0, 0:4])
            nc.gpsimd.collective_compute(
                "AllGather",
                Alu.bypass,
                replica_groups=[[0, 1, 2, 3], [4, 5, 6, 7]],
                ins=[agw_in[:, :]],
                outs=[agw_out[:, :]],
            )

            wqk_sb = const.tile([128, n_dt, 512], bf16)
            nc.sync.dma_start(
                out=wqk_sb, in_=wqk.rearrange("(t p) c -> p t c", p=128)
            )
            bqk_sb = const.tile([128, 4], f32)
            nc.sync.dma_start(out=bqk_sb, in_=bqk[:, :])
            xt_sb = const.tile([128, n_dt, s], bf16)
            xt_r = xt.rearrange("(t p) ss -> p t ss", p=128)

            def load_x(sh):
                for t in range(n_dt):
                    nc.sync.dma_start(
                        out=xt_sb[:, t, sh * IB : (sh + 1) * IB],
                        in_=xt_r[:, t, sh * IB : (sh + 1) * IB],
                    )

            wv_sb = const.tile([128, n_dt, 256], bf16)
            nc.sync.dma_start(
                out=wv_sb, in_=wv.rearrange("(t p) c -> p t c", p=128)
            )
            bv_sb = const.tile([128, 256], f32)
            nc.sync.dma_start(out=bv_sb, in_=bv[:, :])
            load_x(0)
            mskt_sb = const.tile([128, 2, 128], bf16)
            nc.sync.dma_start(out=mskt_sb, in_=mskt[:, :, :])
            mskf_sb = const.tile([128, 4, 2, IB], bf16)
            nc.sync.dma_start(out=mskf_sb, in_=mskf[:, :, :, :])
            sel2_sb = const.tile([1, 64], f32r)
            nc.sync.dma_start(out=sel2_sb, in_=sel2[:, :])
            load_x(1)
            wp_sb = const.tile([128, n_dt, OC], bf16)
            nc.sync.dma_start(
                out=wp_sb, in_=wp.rearrange("(t p) c -> p t c", p=128)
            )
            bpt_sb = const.tile([128, 2], f32)
            nc.sync.dma_start(out=bpt_sb, in_=bpt[:, :])
            load_x(2)
            load_x(3)

            # dummy exp: pulls the ACT exp table load off the critical path
            warm_sb = const.tile([1, 1], f32)
            nc.scalar.activation(
                out=warm_sb, in_=bqk_sb[0:1, 0:1], func=Act.Exp, scale=0.0
            )

            # persistent intermediates (all bf16)
            qT_sb = persist.tile([128, NPAIR, s], bf16)   # [pair_ch, pair, s]
            kT_sb = persist.tile([128, NPAIR, s], bf16)
            v_sb = persist.tile([128, n_st, HLOC, 65], bf16)
            aT_sb = persist.tile([128, NPAIR, s], bf16)

            # ones column for the softmax-denominator trick
            nc.gpsimd.memset(v_sb[:, :, :, 64:65], 1.0)

            ag_in = [
                [dram.tile([128, IB], bf16, name=f"ag_in{i}_{p}") for p in range(2)]
                for i in range(n_ib)
            ]
            ag_out = [
                [dram.tile([512, IB], bf16, name=f"ag_out{i}_{p}") for p in range(2)]
                for i in range(n_ib)
            ]

            def qkT_v(ib):
                # c-tile t: 0,1 = q pair0/1; 2,3 = k pair0/1
                for t in range(4):
                    ps = ps_s.tile([128, IB], f32, name="ps", tag="pss")
                    for dt in range(n_dt):
                        nc.tensor.matmul(
                            ps,
                            lhsT=(wqk_sb[:, dt, t * 128 : (t + 1) * 128]),
                            rhs=(xt_sb[:, dt, ib * IB : (ib + 1) * IB]),
                            start=(dt == 0),
                            stop=(dt == n_dt - 1),
                        )
                    dst = qT_sb if t < 2 else kT_sb
                    nc.vector.tensor_scalar_add(
                        out=dst[:, t % 2, ib * IB : (ib + 1) * IB],
                        in0=ps,
                        scalar1=bqk_sb[:, t : t + 1],
                    )
                for st in range(4 * ib, 4 * ib + 4):
                    psv = ps_s.tile([128, 256], f32, name="psv", tag="pss")
                    for dt in range(n_dt):
                        nc.tensor.matmul(
                            psv,
                            lhsT=(xt_sb[:, dt, st * 128 : (st + 1) * 128]),
                            rhs=(wv_sb[:, dt, :]),
                            start=(dt == 0),
                            stop=(dt == n_dt - 1),
                        )
                    nc.vector.tensor_tensor(
                        out=v_sb[:, st, :, 0:64],
                        in0=psv.rearrange("p (h e) -> p h e", h=HLOC),
                        in1=bv_sb.rearrange("p (h e) -> p h e", h=HLOC),
                        op=Alu.add,
                    )

            def norm_recips(sts):
                """DVE reciprocals of the denominator rows. Emitted at the
                end of an attention block, after its masks, so the next
                block's critical DVE ops never queue behind them."""
                for st_ in sts:
                    for hh in range(2):
                        rec1 = small.tile([1, IB], f32r, name="rec1")
                        with nc.allow_low_precision(
                            reason="float32r feeds the fp32r replicate matmul"
                        ):
                            nc.vector.reciprocal(
                                out=rec1, in_=st_["avs"][hh][64:65, :]
                            )
                        st_["rec1"].append(rec1)

            def norm_repmm(st_, ps_tag="pss"):
                """PE replicate matmuls (emitted as PE filler once the
                reciprocals had time to complete)."""
                for hh in range(2):
                    rec_ps = ps_s.tile([64, IB], f32, name="rec_ps", tag=ps_tag)
                    nc.tensor.matmul(
                        rec_ps,
                        lhsT=(sel2_sb[0:1, 0:64]),
                        rhs=(st_["rec1"][hh][:, :]),
                        start=True,
                        stop=True,
                    )
                    st_["rec_ps"].append(rec_ps)

            def norm_finish(st_):
                """Copies + multiplies + stage + collective for one pair."""
                ib, pair, avs = st_["ib"], st_["pair"], st_["avs"]
                rec_rep = small.tile([128, IB], f32, name="rec_rep")
                for hh in range(2):
                    nc.vector.tensor_copy(
                        out=rec_rep[hh * 64 : (hh + 1) * 64, :],
                        in_=st_["rec_ps"][hh],
                    )
                for hh in range(2):
                    poff = hh * 64
                    nc.vector.tensor_tensor(
                        out=aT_sb[poff : poff + 64, pair, ib * IB : (ib + 1) * IB],
                        in0=avs[hh][0:64, :],
                        in1=rec_rep[poff : poff + 64, :],
                        op=Alu.mult,
                    )
                nc.sync.dma_start(
                    out=ag_in[ib][pair][:, :],
                    in_=aT_sb[:, pair, ib * IB : (ib + 1) * IB],
                )
                nc.gpsimd.collective_compute(
                    "AllGather",
                    Alu.bypass,
                    replica_groups=[[0, 1, 2, 3], [4, 5, 6, 7]],
                    ins=[ag_in[ib][pair][:, :]],
                    outs=[ag_out[ib][pair][:, :]],
                )

            def att_jt(ib, pair, jseq, jt, njt, avs, pend):
                """Emit one jt step (scores, exp, mask, prev AV) for one
                pair; returns the new pending AV pair."""
                k = jt - 4 * ib
                off = 128 * k if (ib > 0 and k >= 1) else 0
                pss = ps_s.tile([128, 2, IB], f32, name="pss", tag="pss")
                for hh in range(2):
                    poff = hh * 64
                    nc.tensor.matmul(
                        pss[:, hh, off:],
                        lhsT=(kT_sb[
                                poff : poff + 64, pair,
                                jt * 128 : (jt + 1) * 128,
                            ]),
                        rhs=(qT_sb[
                                poff : poff + 64, pair,
                                ib * IB + off : (ib + 1) * IB,
                            ]),
                        start=True,
                        stop=True,
                        tile_position=(poff, 0),
                    )
                pt = ptpool.tile([128, 2, IB], bf16, name="pt")
                nc.scalar.activation(
                    out=pt[:, :, off:], in_=pss[:, :, off:],
                    func=Act.Exp, scale=SCALE,
                )
                if k >= 0:  # diagonal tile: causal mask (DVE, bf16 2x mode)
                    if ib > 0 or k == 0:
                        nc.vector.tensor_tensor(
                            out=pt[:, :, off : off + 128],
                            in0=pt[:, :, off : off + 128],
                            in1=mskt_sb,
                            op=Alu.mult,
                        )
                    else:  # ib == 0, k >= 1: full-width mask
                        nc.vector.tensor_tensor(
                            out=pt, in0=pt, in1=mskf_sb[:, k, :, :],
                            op=Alu.mult,
                        )
                if pend is not None:
                    for mm in pend:
                        nc.tensor.matmul(**mm)
                return [
                    dict(
                        out=avs[hh][:, off:],
                        lhsT=(v_sb[:, jt, pair * 2 + hh, :]),
                        rhs=(pt[:, hh, off:]),
                        start=(jseq == 0),
                        stop=(jseq == njt - 1),
                    )
                    for hh in range(2)
                ]

            def attention_both(ib):
                """Both head pairs' jt streams interleaved: one pipeline
                fill per query block instead of two."""
                njt = 4 * (ib + 1)
                jt_order = list(range(4 * ib, njt)) + list(range(4 * ib))
                sts, avs, pend = [], {}, {}
                for pair in range(2):
                    avs[pair] = [
                        ps_av.tile([65, IB], f32, name=f"av{pair}{hh}", tag=f"av{hh}")
                        for hh in range(2)
                    ]
                    pend[pair] = None
                for jseq, jt in enumerate(jt_order):
                    for pair in range(2):
                        pend[pair] = att_jt(
                            ib, pair, jseq, jt, njt, avs[pair], pend[pair]
                        )
                for pair in range(2):
                    for mm in pend[pair]:
                        nc.tensor.matmul(**mm)
                    sts.append(
                        dict(ib=ib, pair=pair, avs=avs[pair], rec1=[], rec_ps=[])
                    )
                norm_recips(sts)
                return sts

            def attention(ib, pair):
                """Single-pair jt stream (used for the final query block so
                pair0's gather can fire during pair1's attention)."""
                njt = 4 * (ib + 1)
                avs = [
                    ps_av.tile([65, IB], f32, name=f"av{hh}", tag=f"av{hh}")
                    for hh in range(2)
                ]
                jt_order = list(range(4 * ib, njt)) + list(range(4 * ib))
                pend = None
                for jseq, jt in enumerate(jt_order):
                    pend = att_jt(ib, pair, jseq, jt, njt, avs, pend)
                for mm in pend:
                    nc.tensor.matmul(**mm)
                st_ = dict(ib=ib, pair=pair, avs=avs, rec1=[], rec_ps=[])
                norm_recips([st_])
                return st_

            def norm_pe(st_, ps_tag="pss"):
                norm_repmm(st_, ps_tag)
                norm_finish(st_)

            def proj_slice(ib):
                """Output projection for gathered slice ib:
                yT[oc, q] = sum_c Wp[c, oc] agT[c, q], accumulating the
                two gathered pair-halves."""
                agfs = []
                for part in range(2):
                    agf_sb = agfpool.tile(
                        [128, 4, IB], bf16, name=f"agf{part}", tag="agf"
                    )
                    nc.sync.dma_start(
                        out=agf_sb,
                        in_=ag_out[ib][part].rearrange("(t p) q -> p t q", p=128),
                    )
                    agfs.append(agf_sb)
                psy = [
                    ps_s.tile([128, IB], f32, name=f"psy{h}", tag="pss")
                    for h in range(2)
                ]
                for part in range(2):
                    for h in range(2):
                        for g in range(4):
                            nc.tensor.matmul(
                                psy[h],
                                lhsT=(wp_sb[:, 2 * g + part, h * 128 : (h + 1) * 128]),
                                rhs=(agfs[part][:, g, :]),
                                start=(part == 0 and g == 0),
                                stop=(part == 1 and g == 3),
                            )
                for h in range(2):
                    ysb = yout.tile([128, IB], f32, name="ysb")
                    nc.vector.tensor_scalar_add(
                        out=ysb, in0=psy[h], scalar1=bpt_sb[:, h : h + 1]
                    )
                    nc.sync.dma_start(
                        out=y[h * 128 : (h + 1) * 128, ib * IB : (ib + 1) * IB],
                        in_=ysb,
                    )

            # Iterations 0..n-2: pairs interleaved; both normalizes deferred
            # behind the next iteration's qkT/v block (the reciprocals run
            # during it, so the PE replicate matmuls never stall).
            # Final iteration: pairs sequential so pair0's gather flies
            # during pair1's attention; proj of the previous slice fills
            # the reciprocal latency.
            pendings = []
            for ib in range(n_ib - 1):
                qkT_v(ib)
                for st_ in pendings:
                    norm_pe(st_)
                pendings = attention_both(ib)
                if ib > 0:
                    proj_slice(ib - 1)
            ib = n_ib - 1
            qkT_v(ib)
            for st_ in pendings:
                norm_pe(st_)
            st0 = attention(ib, 0)
            proj_slice(ib - 1)      # fills pair0's reciprocal latency
            norm_pe(st0)            # fires AG(last, pair0) before att pair1
            st1 = attention(ib, 1)
            norm_pe(st1)
            proj_slice(ib)

    nc.compile()
    return nc


def _shard_inputs(x, w_attn, b_attn, w_proj, b_proj, s=S):
    """Host-side sharding: build the per-core input maps."""
    import ml_dtypes
    bf16 = ml_dtypes.bfloat16
    x = np.asarray(x, dtype=np.float32)
    w_attn = np.asarray(w_attn, dtype=np.float32)
    b_attn = np.asarray(b_attn, dtype=np.float32)
    w_proj = np.asarray(w_proj, dtype=np.float32)
    b_proj = np.asarray(b_proj, dtype=np.float32)

    # full-width causal mask tiles for ib=0: mskf[j, k, :, i] = 1.0 if i >= j+128k
    jj = np.arange(128)[:, None, None, None]
    kk = np.arange(4)[None, :, None, None]
    ii = np.arange(IB)[None, None, None, :]
    mskf = np.broadcast_to(ii >= jj + 128 * kk, (128, 4, 2, IB)).astype(bf16)
    # triangle mask (duplicated for the 2-head layout)
    mskt = np.broadcast_to(
        (np.arange(128)[None, None, :] >= np.arange(128)[:, None, None]),
        (128, 2, 128),
    ).astype(bf16)

    in_maps = []
    for core in range(N_CORES):
        b, g = divmod(core, GROUP)
        hs = list(range(g * HLOC, (g + 1) * HLOC))
        xt = np.ascontiguousarray(x[b].T).astype(bf16)
        qcols = np.concatenate(
            [w_attn[:, h * HD : (h + 1) * HD] for h in hs], axis=1
        )
        kcols = np.concatenate(
            [w_attn[:, D + h * HD : D + (h + 1) * HD] for h in hs], axis=1
        )
        vcols = np.concatenate(
            [w_attn[:, 2 * D + h * HD : 2 * D + (h + 1) * HD] for h in hs], axis=1
        )
        wqk = np.ascontiguousarray(
            np.concatenate([qcols, kcols], axis=1)
        ).astype(bf16)
        wv = np.ascontiguousarray(vcols).astype(bf16)
        bq = np.concatenate([b_attn[h * HD : (h + 1) * HD] for h in hs])
        bk = np.concatenate([b_attn[D + h * HD : D + (h + 1) * HD] for h in hs])
        bvv = np.concatenate(
            [b_attn[2 * D + h * HD : 2 * D + (h + 1) * HD] for h in hs]
        )
        bqk = np.concatenate([bq, bk]).reshape(4, 128).T.copy()  # [128, 4]
        bv = np.broadcast_to(bvv, (128, 256)).copy()
        wpc = np.ascontiguousarray(w_proj[:, g * OC : (g + 1) * OC]).astype(bf16)
        bpt = np.ascontiguousarray(
            b_proj[g * OC : (g + 1) * OC].reshape(2, 128).T
        )  # bpt[p, h] = b_proj[g*256 + 128h + p]
        in_maps.append(
            dict(
                xt=xt, wqk=wqk, wv=wv, bqk=bqk, bv=bv, wp=wpc, bpt=bpt,
                mskf=mskf, mskt=mskt, sel2=np.ones((1, 64), np.float32),
            )
        )
    return in_maps


def _unshard(results):
    y = np.empty((B, S, D), np.float32)
    for core in range(N_CORES):
        b, g = divmod(core, GROUP)
        y[b, :, g * OC : (g + 1) * OC] = results[core]["y"].T
    return y


_NC_CACHE = {}


def kernel(x, w_attn, b_attn, w_proj, b_proj):
    from concourse.bass_utils import run_bass_kernel_spmd

    if S not in _NC_CACHE:
        _NC_CACHE[S] = _build_bass(S)
    nc = _NC_CACHE[S]
    in_maps = _shard_inputs(x, w_attn, b_attn, w_proj, b_proj)
    res = run_bass_kernel_spmd(nc, in_maps, list(range(N_CORES)))
    return _unshard(res.results)
